# revision 1
# baseline (speedup 1.0000x reference)
"""Polyakov-loop generator kernel for Trainium2 (8 NeuronCores, SPMD).

Problem: U (complex 3x3 link field) on a [4, 24,24,24,24] lattice.
For each direction mu, every site x: P(x) = prod_{k=0..23} U_mu(x + k*mu_hat)
(periodic).  Output = stack([Re, Im]) of shape [2, 4, 24,24,24,24, 3, 3].

Strategy:
  - core c in 0..7 handles direction mu = c // 2, half h = c % 2 of the
    13824-site batch (the three non-mu lattice axes).
  - host canonicalizes U[mu] to [24 (mu-axis slice k), 6912 sites, 9 entries]
    per core, so the SPMD program is direction-agnostic.
  - on-chip: all 24 cyclic products per line via prefix/suffix products:
        Q(s) = V0..V(s-1),  S(s) = Vs..V23,  P(s) = S(s) @ Q(s)
    68 batched complex-3x3 matmul steps instead of the naive 23*24.
  - each matmul step: 22 DVE tensor_tensor ops over [128, 54*9] fp32 tiles
    using broadcast (step-0) access patterns fusing all 9 output entries.
"""

import sys
import types

sys.path.insert(0, "/opt/trn_rl_repo")

import numpy as np

import concourse.bacc as bacc
import concourse.mybir as mybir
from concourse.tile import TileContext
from concourse.bass_utils import run_bass_kernel_spmd

F32 = mybir.dt.float32
L = 24          # lattice extent (product length)
NMU = 4
P = 128         # SBUF partitions
F = 54          # site-columns per partition (6912 = 128 * 54)
E = 9           # 3x3 entries
FB = F * E      # 486 free elems per slice
HALF = P * F    # 6912 sites per core
MULT = mybir.AluOpType.mult
ADD = mybir.AluOpType.add
SUB = mybir.AluOpType.subtract

_prog_cache = {}


def _v4(ap_flat):
    """[128, FB] flat AP -> [128, 54(s), 3(i), 3(j)] view."""
    return ap_flat.rearrange("p (s i j) -> p s i j", s=F, i=3, j=3)


def _a_bcast(x4, j):
    """A[:, s, :, j] broadcast over output k -> [128, 54, 3, 3]."""
    return x4[:, :, :, j].unsqueeze(3).broadcast_to([P, F, 3, 3])


def _b_bcast(x4, j):
    """B[:, s, j, :] broadcast over output i -> [128, 54, 3, 3]."""
    return x4[:, :, j, :].unsqueeze(2).broadcast_to([P, F, 3, 3])


def _cmatmul(nc, tmp_pool, cre, cim, are, aim, bre, bim):
    """C = A @ B, batched complex 3x3 over [128, FB] fp32 SBUF APs.

    cre/cim must be distinct tiles from are/aim/bre/bim.
    """
    tt = nc.vector.tensor_tensor
    c_re, c_im = _v4(cre), _v4(cim)
    a_re, a_im = _v4(are), _v4(aim)
    b_re, b_im = _v4(bre), _v4(bim)
    t = tmp_pool.tile([P, FB], F32, tag="tmp")
    t4 = _v4(t[:])
    for j in range(3):
        ar, ai = _a_bcast(a_re, j), _a_bcast(a_im, j)
        br, bi = _b_bcast(b_re, j), _b_bcast(b_im, j)
        if j == 0:
            tt(out=c_re, in0=ar, in1=br, op=MULT)
            tt(out=t4, in0=ai, in1=bi, op=MULT)
            tt(out=c_re, in0=c_re, in1=t4, op=SUB)
            tt(out=c_im, in0=ar, in1=bi, op=MULT)
            tt(out=t4, in0=ai, in1=br, op=MULT)
            tt(out=c_im, in0=c_im, in1=t4, op=ADD)
        else:
            tt(out=t4, in0=ar, in1=br, op=MULT)
            tt(out=c_re, in0=c_re, in1=t4, op=ADD)
            tt(out=t4, in0=ai, in1=bi, op=MULT)
            tt(out=c_re, in0=c_re, in1=t4, op=SUB)
            tt(out=t4, in0=ar, in1=bi, op=MULT)
            tt(out=c_im, in0=c_im, in1=t4, op=ADD)
            tt(out=t4, in0=ai, in1=br, op=MULT)
            tt(out=c_im, in0=c_im, in1=t4, op=ADD)


def build_program():
    if "nc" in _prog_cache:
        return _prog_cache["nc"]

    nc = bacc.Bacc("TRN2", target_bir_lowering=False, debug=False, num_devices=8)
    ure_d = nc.declare_dram_parameter("u_re", [L, P, FB], F32, isOutput=False)
    uim_d = nc.declare_dram_parameter("u_im", [L, P, FB], F32, isOutput=False)
    pre_d = nc.declare_dram_parameter("p_re", [L, P, FB], F32, isOutput=True)
    pim_d = nc.declare_dram_parameter("p_im", [L, P, FB], F32, isOutput=True)

    with TileContext(nc) as tc:
        with (
            tc.tile_pool(name="qpool", bufs=1) as qpool,
            tc.tile_pool(name="upool", bufs=6) as upool,
            tc.tile_pool(name="spool", bufs=3) as spool,
            tc.tile_pool(name="ppool", bufs=4) as ppool,
            tc.tile_pool(name="tpool", bufs=4) as tpool,
        ):
            # Persistent prefix store: Q(s) for s = 2..23 at slot s-2.
            q_re = qpool.tile([P, 22 * FB], F32, tag="q_re")
            q_im = qpool.tile([P, 22 * FB], F32, tag="q_im")

            def qsl(s):
                lo = (s - 2) * FB
                return q_re[:, lo:lo + FB], q_im[:, lo:lo + FB]

            def load_u(k):
                ur = upool.tile([P, FB], F32, tag="u_re")
                ui = upool.tile([P, FB], F32, tag="u_im")
                nc.sync.dma_start(out=ur[:], in_=ure_d[k])
                nc.sync.dma_start(out=ui[:], in_=uim_d[k])
                return ur[:], ui[:]

            def store_p(k, pr, pi):
                nc.sync.dma_start(out=pre_d[k], in_=pr)
                nc.sync.dma_start(out=pim_d[k], in_=pi)

            # ---- prefix pass: Q(k+1) = Q(k) @ V(k) ----
            cur = None
            for k in range(L):
                u = load_u(k)
                if k == 0:
                    cur = u                     # Q(1) = V0 (alias)
                    continue
                if k <= L - 2:
                    dst = qsl(k + 1)            # Q(2)..Q(23)
                else:
                    prt = ppool.tile([P, FB], F32, tag="p_re")
                    pit = ppool.tile([P, FB], F32, tag="p_im")
                    dst = (prt[:], pit[:])      # Q(24) = P(0)
                _cmatmul(nc, tpool, dst[0], dst[1], cur[0], cur[1], u[0], u[1])
                if k == L - 1:
                    store_p(0, dst[0], dst[1])
                cur = dst

            # ---- suffix pass: S(k) = V(k) @ S(k+1); P(k) = S(k) @ Q(k) ----
            s_cur = None
            for k in range(L - 1, -1, -1):
                u = load_u(k)
                if k == L - 1:
                    s_cur = u                   # S(23) = V23 (alias)
                    qs = qsl(k)
                    prt = ppool.tile([P, FB], F32, tag="p_re")
                    pit = ppool.tile([P, FB], F32, tag="p_im")
                    _cmatmul(nc, tpool, prt[:], pit[:],
                             s_cur[0], s_cur[1], qs[0], qs[1])
                    store_p(k, prt[:], pit[:])
                elif k >= 1:
                    sr = spool.tile([P, FB], F32, tag="s_re")
                    si = spool.tile([P, FB], F32, tag="s_im")
                    _cmatmul(nc, tpool, sr[:], si[:],
                             u[0], u[1], s_cur[0], s_cur[1])
                    s_cur = (sr[:], si[:])
                    if k >= 2:
                        qs = qsl(k)
                        prt = ppool.tile([P, FB], F32, tag="p_re")
                        pit = ppool.tile([P, FB], F32, tag="p_im")
                        _cmatmul(nc, tpool, prt[:], pit[:],
                                 s_cur[0], s_cur[1], qs[0], qs[1])
                        store_p(k, prt[:], pit[:])
                else:
                    # k == 0: P(1) = S(1) @ Q(1), Q(1) = V0
                    prt = ppool.tile([P, FB], F32, tag="p_re")
                    pit = ppool.tile([P, FB], F32, tag="p_im")
                    _cmatmul(nc, tpool, prt[:], pit[:],
                             s_cur[0], s_cur[1], u[0], u[1])
                    store_p(1, prt[:], pit[:])

    nc.compile()
    _prog_cache["nc"] = nc
    return nc


def _canonicalize(U_re, U_im):
    """Full inputs -> per-core input maps (core c: mu = c//2, half = c%2)."""
    in_maps = []
    for c in range(8):
        mu, h = c // 2, c % 2
        m = {}
        for name, U in (("u_re", U_re), ("u_im", U_im)):
            canon = np.moveaxis(U[mu], mu, 0).reshape(L, L**3, E)
            shard = canon[:, h * HALF:(h + 1) * HALF, :]
            m[name] = np.ascontiguousarray(shard).reshape(L, P, FB)
        in_maps.append(m)
    return in_maps


def _assemble(results):
    out = np.empty((2, NMU, L, L, L, L, 3, 3), dtype=np.float32)
    for mu in range(4):
        for ri, name in ((0, "p_re"), (1, "p_im")):
            halves = [results[2 * mu + h][name].reshape(L, HALF, E)
                      for h in (0, 1)]
            canon = np.concatenate(halves, axis=1)          # [24, 13824, 9]
            rest = [d for d in range(4) if d != mu]
            shape = (L,) + tuple(L for _ in rest) + (3, 3)
            arr = canon.reshape(shape)                      # [k, b0,b1,b2, 3,3]
            out[ri, mu] = np.moveaxis(arr, 0, mu)
    return out


def kernel(U_re, U_im):
    U_re = np.asarray(U_re, dtype=np.float32)
    U_im = np.asarray(U_im, dtype=np.float32)
    nc = build_program()
    in_maps = _canonicalize(U_re, U_im)
    res = run_bass_kernel_spmd(nc, in_maps, core_ids=list(range(8)))
    return _assemble(res.results)
